# revision 27
# baseline (speedup 1.0000x reference)
"""CCAMDec (channel-attention decoder) Trainium2 Bass kernel.

Data-parallel over batch N=8 across 8 NeuronCores (one batch per core).
Per core (C=512, K=64, HW=4096):
  energy[c,k]   = sum_s x[c,s] * y[k,s]         (bf16 matmul, fp32 accum)
  att[c,k]      = softmax_k(max_k(E) - E)       (== exp(min_k(E)-E)/sum)
  out[c,s]      = x[c,s] + scale * sum_k att[c,k] y[k,s]

All tensors cross HBM in bf16 with the s-contraction layout prepared on
the host (DMA is the roofline: ~9.4MB/core at ~380GB/s):
  xT  [128, 32*512]: chunk g cols [512g:512g+512] = x[:, 128g+p].T
  yT  [128, 32*64]:  chunk g cols [64g:64g+64]    = y[:, 128g+p].T
  y   [64, 4096]:    normal layout (out-matmul weights); ordered AFTER x
                     on the DMA ring so its half-rate 64-partition drain
                     hides inside the softmax window
  out [2048, 1024]:  rows [128q:128q+128] cols 512j+c = outT[128(2q+j)+p, c]
Energy accumulates as e^T[64k, 512c] (yT chunks stationary, xT streamed),
so no on-chip transposes of x or y are needed.  The residual is exact
for scale=0: scale folds into the attention weights, and the output
equals bf16(x) bit-for-bit (identity-matmul or DVE add of +0.0 onto the
loaded xT).
"""

import numpy as np
import ml_dtypes

N, C, K, H, W = 8, 512, 64, 64, 64
S = H * W          # 4096
SC = S // 128      # 32 s-chunks of 128
CC = C // 128      # 4 c-chunks of 128
BF = ml_dtypes.bfloat16

_CACHE = {}


def _pack_xT(x):
    # x [n, C, S] -> [n, 128, SC*512] bf16; chunk g cols = x[:, 128g+p].T
    n = x.shape[0]
    xb = x.reshape(n, C, SC, 128).astype(BF)
    return np.ascontiguousarray(xb.transpose(0, 3, 2, 1)).reshape(n, 128, SC * C)


def _pack_yT(y):
    # y [n, K, S] -> [n, 128, SC*64] bf16
    n = y.shape[0]
    yb = y.reshape(n, K, SC, 128).astype(BF)
    return np.ascontiguousarray(yb.transpose(0, 3, 2, 1)).reshape(n, 128, SC * K)


def _pack_y(y):
    return np.ascontiguousarray(y.astype(BF))


def _unpack_out(o):
    # o [n, 1024, 2048] bf16 -> [n, C, S] fp32
    n = o.shape[0]
    o = o.reshape(n, 8, 128, 4, 512).transpose(0, 1, 3, 2, 4).reshape(n, S, C)
    return np.ascontiguousarray(o.transpose(0, 2, 1)).astype(np.float32)


def _build_program():
    import concourse.tile as tile
    from concourse import bacc, mybir
    from concourse.masks import make_identity

    F32 = mybir.dt.float32
    BF16 = mybir.dt.bfloat16
    AX = mybir.AxisListType
    OP = mybir.AluOpType
    AF = mybir.ActivationFunctionType

    nc = bacc.Bacc("TRN2", target_bir_lowering=False, debug=False)
    xT_d = nc.dram_tensor("xT", [128, SC * 512], BF16, kind="ExternalInput")
    yT_d = nc.dram_tensor("yT", [128, SC * 64], BF16, kind="ExternalInput")
    y_d = nc.dram_tensor("y", [K, S], BF16, kind="ExternalInput")
    s_d = nc.dram_tensor("scale", [1], F32, kind="ExternalInput")
    o_d = nc.dram_tensor("out", [S // 4, 2048], BF16, kind="ExternalOutput")

    with tile.TileContext(nc) as tc:
        with (
            tc.tile_pool(name="const", bufs=1) as const,
            tc.tile_pool(name="xp", bufs=1) as xp,
            tc.tile_pool(name="ytp", bufs=1) as ytp,
            tc.tile_pool(name="ysbp", bufs=1) as ysbp,
            tc.tile_pool(name="etp", bufs=1) as etp,
            tc.tile_pool(name="smp", bufs=12) as smp,
            tc.tile_pool(name="pp", bufs=4) as pp,
            tc.tile_pool(name="attp", bufs=1) as attp,
            tc.tile_pool(name="resp", bufs=6) as resp,
        ):
            # DMA order on the HWDGE ring: yT (feeds the energy weights),
            # x ramping big then small (fine arrival granularity for the
            # last energy matmuls), y-normal last (needed only at the out
            # phase; overlaps the softmax window).
            yT_all = ytp.tile([128, SC * 64], BF16)
            nc.scalar.dma_start(out=yT_all, in_=yT_d[:])
            xT_all = xp.tile([128, SC * 512], BF16)
            splits = [8, 20, 4]  # in 512-col units; wide chunks keep
            # per-partition descriptor runs long (HBM reads pay a
            # ~150ns per-descriptor turnaround, so short runs starve)
            a = 0
            for w in splits:
                nc.sync.dma_start(
                    out=xT_all[:, a * 512 : (a + w) * 512],
                    in_=xT_d[:, a * 512 : (a + w) * 512],
                )
                a += w
            # y and attT are zero-padded to 128 partitions so the out-
            # matmuls contract over the full PE array (P=64 activity does
            # not engage the HAM clock-unthrottle; rows 64-127 are zero
            # and contribute nothing)
            y_sb = ysbp.tile([128, S], BF16)
            nc.gpsimd.memset(y_sb[K:128, :], 0.0)
            nc.scalar.dma_start(out=y_sb[0:K, :], in_=y_d[:])

            ident = const.tile([128, 128], BF16)
            make_identity(nc, ident)
            ident_f = const.tile([128, 128], F32)
            make_identity(nc, ident_f)

            scale_sb = const.tile([128, 1], F32)
            nc.gpsimd.dma_start(out=scale_sb, in_=s_d[:].to_broadcast([128, 1]))

            # prewarm both ScalarE LUTs (Exp and Copy) in the DMA-ramp head
            warm_in = const.tile([128, 1], F32)
            nc.vector.memset(warm_in, 0.0)
            warm = const.tile([128, 1], F32)
            nc.scalar.activation(out=warm, in_=warm_in, func=AF.Exp)
            warm2 = const.tile([128, 1], F32)
            nc.scalar.activation(out=warm2, in_=warm_in, func=AF.Copy)

            wa = const.tile([128, 128], BF16)
            nc.vector.memset(wa, 0.0)
            wa_f = const.tile([128, 128], F32)
            nc.vector.memset(wa_f, 0.0)

            attT_sb = attp.tile([128, C], BF16)
            nc.gpsimd.memset(attT_sb[K:128, :], 0.0)
            with (
                tc.tile_pool(name="wu_ps", bufs=1, space="PSUM") as wu_ps,
                tc.tile_pool(name="e_ps", bufs=1, space="PSUM") as e_ps,
                tc.tile_pool(name="sm_ps", bufs=1, space="PSUM") as sm_ps,
                tc.tile_pool(name="at_ps", bufs=1, space="PSUM") as at_ps,
                tc.tile_pool(name="o_ps", bufs=4, space="PSUM") as o_ps,
            ):
                # dummy-matmul bursts trip the PE HAM activity monitor to
                # K=8/8 (2.4GHz) during the DMA ramp, and keep it there
                # through the softmax latency chain
                wp = wu_ps.tile([128, 128], F32)

                # keep-warm matmuls: dummy PE work, where needed pinned
                # behind real tiles so the Tile scheduler cannot hoist
                # them out of the phase they keep the HAM busy-window
                # alive in (idle >3.4us re-throttles PE to 1.2GHz)
                def keep_warm(n):
                    for _ in range(n):
                        nc.tensor.matmul(
                            wp[:], lhsT=wa[:], rhs=wa[:], start=True, stop=True
                        )

                keep_warm(6)

                # energy: e^T[64k, 512c] accumulated over the 32 s-chunks;
                # yT chunk is the stationary operand, xT streams 512 cols
                e_t = e_ps.tile([K, C], F32)
                for g in range(SC):
                    nc.tensor.matmul(
                        e_t[:],
                        lhsT=yT_all[:, g * 64 : (g + 1) * 64],
                        rhs=xT_all[:, g * 512 : (g + 1) * 512],
                        start=(g == 0),
                        stop=(g == SC - 1),
                    )
                    if g % 4 == 1:
                        # insurance against HAM idle-windows while the
                        # early DMA stream ramps: pinned behind chunk g
                        nc.tensor.matmul(
                            wp[:],
                            lhsT=wa[:],
                            rhs=xT_all[:, g * 512 : g * 512 + 128],
                            start=True,
                            stop=True,
                        )

                # softmax, wave-pipelined across the four c-chunks:
                # E^T -> SBUF (per-cc ScalarE copies), PE transpose to
                # [c,k], DVE min, ScalarE Exp(bias=min, accum sum) +
                # reciprocal (same queue, no cross-engine hop), DVE
                # (p*rcp*scale) -> bf16, PE transpose back, DVE copy to
                # attT slice.  scale folds into the weights here.
                eT_sb = etp.tile([K, C], F32)
                for cc in range(CC):
                    nc.scalar.activation(
                        out=eT_sb[:, cc * 128 : (cc + 1) * 128],
                        in_=e_t[:, cc * 128 : (cc + 1) * 128],
                        func=AF.Copy,
                    )
                ecc_all = sm_ps.tile([128, CC * K], F32, tag="sm")
                eccs = [ecc_all[:, cc * K : (cc + 1) * K] for cc in range(CC)]
                for cc in range(CC):
                    nc.tensor.transpose(
                        eccs[cc], eT_sb[:, cc * 128 : (cc + 1) * 128], ident_f[0:K, 0:K]
                    )
                for cc in range(CC):
                    # pinned behind eT_sb: fills the PE idle window while
                    # the DVE/ScalarE softmax chain runs
                    nc.tensor.matmul(
                        wp[:],
                        lhsT=wa_f[0:K, :],
                        rhs=eT_sb[:, cc * 128 : (cc + 1) * 128],
                        start=True,
                        stop=True,
                    )
                rmins = []
                for cc in range(CC):
                    rmin = smp.tile([128, 1], F32, tag="sm")
                    nc.vector.tensor_reduce(
                        out=rmin, in_=eccs[cc], axis=AX.X, op=OP.min
                    )
                    rmins.append(rmin)
                rcps, p_ts = [], []
                for cc in range(CC):
                    p_t = pp.tile([128, K], F32, tag="p")
                    ssum = smp.tile([128, 1], F32, tag="sm")
                    nc.scalar.activation(
                        out=p_t[:],
                        in_=eccs[cc],
                        func=AF.Exp,
                        bias=rmins[cc],
                        scale=-1.0,
                        accum_out=ssum,
                    )
                    rcp = smp.tile([128, 1], F32, tag="sm")
                    nc.vector.reciprocal(out=rcp, in_=ssum)
                    p_ts.append(p_t)
                    rcps.append(rcp)
                for cc in range(CC):
                    nc.tensor.matmul(
                        wp[:, 0:K],
                        lhsT=wa_f[:],
                        rhs=p_ts[cc][:],
                        start=True,
                        stop=True,
                    )
                at_all = at_ps.tile([K, C], BF16, tag="at")
                for cc in range(CC):
                    attbf = pp.tile([128, K], BF16, tag="att")
                    nc.vector.tensor_scalar(
                        out=attbf[:],
                        in0=p_ts[cc][:],
                        scalar1=rcps[cc],
                        scalar2=scale_sb,
                        op0=OP.mult,
                        op1=OP.mult,
                    )
                    nc.tensor.matmul(
                        wp[:, 0:K], lhsT=wa[:], rhs=attbf[:], start=True, stop=True
                    )
                    nc.tensor.transpose(at_all[:, cc * 128 : (cc + 1) * 128], attbf[:], ident)
                    nc.scalar.activation(
                        out=attT_sb[0:K, cc * 128 : (cc + 1) * 128],
                        in_=at_all[:, cc * 128 : (cc + 1) * 128],
                        func=AF.Copy,
                    )
                    nc.tensor.matmul(
                        wp[:],
                        lhsT=wa[0:K, :],
                        rhs=attT_sb[0:K, cc * 128 : (cc + 1) * 128],
                        start=True,
                        stop=True,
                    )

                # out phase: outT[128s, 512c] = y_g^T @ att^T + xT_g.
                # per-chunk residual engine: V chunks via DVE add from
                # PSUM, P chunks via PE identity-matmul accumulate +
                # ScalarE copy (16 each).  The first 8 chunks run
                # cc-sliced (N=128 matmuls per attT slice) so they start
                # inside the softmax latency chain as attT slices land.
                # Stores are 512KB groups of 4 chunks.
                mode = ["V", "P"] * (SC // 2)
                for q4 in range(SC // 4):
                    res = resp.tile([128, 2048], BF16, name=f"res{q4}", tag="res")
                    for j in range(4):
                        g = 4 * q4 + j
                        o_t = o_ps.tile([128, C], F32, name=f"o_t{g}", tag="o_t")
                        sliced = g < 8 and mode[g] == "V"
                        if sliced:
                            for cc in range(CC):
                                nc.tensor.matmul(
                                    o_t[:, cc * 128 : (cc + 1) * 128],
                                    lhsT=y_sb[:, g * 128 : (g + 1) * 128],
                                    rhs=attT_sb[:, cc * 128 : (cc + 1) * 128],
                                    start=True,
                                    stop=True,
                                )
                        else:
                            nc.tensor.matmul(
                                o_t[:],
                                lhsT=y_sb[:, g * 128 : (g + 1) * 128],
                                rhs=attT_sb[:],
                                start=True,
                                stop=(mode[g] == "V"),
                            )
                        if mode[g] == "V":
                            nc.vector.tensor_add(
                                res[:, j * 512 : (j + 1) * 512],
                                xT_all[:, g * 512 : (g + 1) * 512],
                                o_t[:],
                            )
                        else:
                            nc.tensor.matmul(
                                o_t[:],
                                lhsT=ident,
                                rhs=xT_all[:, g * 512 : (g + 1) * 512],
                                start=False,
                                stop=True,
                            )
                            nc.scalar.activation(
                                out=res[:, j * 512 : (j + 1) * 512],
                                in_=o_t[:],
                                func=AF.Copy,
                            )
                    nc.sync.dma_start(
                        out=o_d[q4 * 128 : (q4 + 1) * 128, :], in_=res[:]
                    )
    nc.compile()
    return nc


def _get_program():
    if "nc" not in _CACHE:
        _CACHE["nc"] = _build_program()
    return _CACHE["nc"]


def kernel(x, y, scale):
    from concourse import bass2jax

    nc = _get_program()
    x = np.asarray(x, dtype=np.float32).reshape(N, C, S)
    y = np.asarray(y, dtype=np.float32).reshape(N, K, S)
    scale = np.ascontiguousarray(np.asarray(scale, dtype=np.float32)).reshape(1)

    xT = _pack_xT(x)
    yT = _pack_yT(y)
    yn = _pack_y(y)
    in_maps = [
        {"xT": xT[i], "yT": yT[i], "y": yn[i], "scale": scale} for i in range(N)
    ]
    results = bass2jax.run_bass_via_pjrt(nc, in_maps, n_cores=N)
    o = np.stack([np.asarray(results[i]["out"]) for i in range(N)])
    return _unpack_out(o).reshape(N, C, H, W)


# revision 28
# speedup vs baseline: 1.1193x; 1.1193x over previous
"""CCAMDec (channel-attention decoder) Trainium2 Bass kernel.

Data-parallel over batch N=8 across 8 NeuronCores (one batch per core).
Per core (C=512, K=64, HW=4096):
  energy[c,k]   = sum_s x[c,s] * y[k,s]         (bf16 matmul, fp32 accum)
  att[c,k]      = softmax_k(max_k(E) - E)       (== exp(min_k(E)-E)/sum)
  out[c,s]      = x[c,s] + scale * sum_k att[c,k] y[k,s]

Everything crosses HBM in bf16 with the s-contraction layout prepared on
the host (DMA is the roofline; ~8.9MB/core):
  xyT [128, 2048+16384]: cols [64g:64g+64]         = y[:, 128g+p].T (yT)
                         cols [2048+512g:...+512]  = x[:, 128g+p].T (xT)
  out [1024, 2048]: rows [128q:128q+128] cols 512j+c = outT[128(4q+j)+p, c]
One input tensor, loaded in three wide column chunks: HBM *reads* pay a
~150ns per-descriptor turnaround, so per-partition descriptor runs must
be long (12/16/8KB here ~= 75-83% of line rate; a 1KB-run tail chunk
would run at ~20%).

Energy accumulates as e^T[64k, 512c] (yT chunks stationary, xT streamed)
so no transposes sit on the load critical path.  y's [64,128] out-matmul
weight tiles are PE-transposed from yT during the load window (8 per
PSUM bank, one DVE copy per bank).  y/attT are zero-padded to 128
partitions: P=64 matmuls do not engage the PE HAM clock-unthrottle, and
a 1.2GHz out phase costs ~10us.  Dummy keep-warm matmuls pinned behind
softmax tiles bridge the HAM busy-window through the softmax latency
chain, and the P-chunk identity-matmuls (residual add of xT into PSUM,
commutative with the attention matmul) pre-run there too.

The residual is exact for scale=0: scale folds into the attention
weights, and the output equals bf16(x) bit-for-bit.
"""

import numpy as np
import ml_dtypes

N, C, K, H, W = 8, 512, 64, 64, 64
S = H * W          # 4096
SC = S // 128      # 32 s-chunks of 128
CC = C // 128      # 4 c-chunks of 128
YW = SC * K        # 2048 cols of yT at the head of xyT
BF = ml_dtypes.bfloat16

_CACHE = {}


def _pack_inputs(x, y):
    # x [n, C, S], y [n, K, S] -> xyT [n, 128, YW + SC*512] bf16
    n = x.shape[0]
    xb = x.reshape(n, C, SC, 128).astype(BF)
    xT = np.ascontiguousarray(xb.transpose(0, 3, 2, 1)).reshape(n, 128, SC * C)
    yb = y.reshape(n, K, SC, 128).astype(BF)
    yT = np.ascontiguousarray(yb.transpose(0, 3, 2, 1)).reshape(n, 128, YW)
    return np.concatenate([yT, xT], axis=2)


def _unpack_out(o):
    # o [n, 1024, 2048] bf16 -> [n, C, S] fp32
    n = o.shape[0]
    o = o.reshape(n, 8, 128, 4, 512).transpose(0, 1, 3, 2, 4).reshape(n, S, C)
    return np.ascontiguousarray(o.transpose(0, 2, 1)).astype(np.float32)


def _build_program():
    import concourse.tile as tile
    from concourse import bacc, mybir
    from concourse.masks import make_identity

    F32 = mybir.dt.float32
    BF16 = mybir.dt.bfloat16
    AX = mybir.AxisListType
    OP = mybir.AluOpType
    AF = mybir.ActivationFunctionType

    nc = bacc.Bacc("TRN2", target_bir_lowering=False, debug=False)
    xy_d = nc.dram_tensor("xyT", [128, YW + SC * 512], BF16, kind="ExternalInput")
    s_d = nc.dram_tensor("scale", [1], F32, kind="ExternalInput")
    o_d = nc.dram_tensor("out", [S // 4, 2048], BF16, kind="ExternalOutput")

    with tile.TileContext(nc) as tc:
        with (
            tc.tile_pool(name="const", bufs=1) as const,
            tc.tile_pool(name="xyp", bufs=1) as xyp,
            tc.tile_pool(name="ysbp", bufs=1) as ysbp,
            tc.tile_pool(name="etp", bufs=1) as etp,
            tc.tile_pool(name="smp", bufs=12) as smp,
            tc.tile_pool(name="pp", bufs=4) as pp,
            tc.tile_pool(name="attp", bufs=1) as attp,
            tc.tile_pool(name="resp", bufs=4) as resp,
        ):
            xy = xyp.tile([128, YW + SC * 512], BF16)
            for a, b in ((0, 6144), (6144, 14336), (14336, YW + SC * 512)):
                nc.sync.dma_start(out=xy[:, a:b], in_=xy_d[:, a:b])

            def xT(g):
                return xy[:, YW + g * 512 : YW + (g + 1) * 512]

            def yT(g):
                return xy[:, g * 64 : (g + 1) * 64]

            ident = const.tile([128, 128], BF16)
            make_identity(nc, ident)
            ident_f = const.tile([128, 128], F32)
            make_identity(nc, ident_f)

            scale_sb = const.tile([128, 1], F32)
            nc.gpsimd.dma_start(out=scale_sb, in_=s_d[:].to_broadcast([128, 1]))

            # prewarm both ScalarE LUTs (Exp and Copy) in the DMA-ramp head
            warm_in = const.tile([128, 1], F32)
            nc.vector.memset(warm_in, 0.0)
            warm = const.tile([128, 1], F32)
            nc.scalar.activation(out=warm, in_=warm_in, func=AF.Exp)
            warm2 = const.tile([128, 1], F32)
            nc.scalar.activation(out=warm2, in_=warm_in, func=AF.Copy)

            wa = const.tile([128, 128], BF16)
            nc.vector.memset(wa, 0.0)
            wa_f = const.tile([128, 128], F32)
            nc.vector.memset(wa_f, 0.0)

            # y and attT zero-padded to 128 partitions (P=128 matmuls keep
            # the PE HAM engaged; rows 64-127 contribute nothing)
            y_sb = ysbp.tile([128, S], BF16)
            nc.gpsimd.memset(y_sb[K:128, :], 0.0)
            attT_sb = attp.tile([128, C], BF16)
            nc.gpsimd.memset(attT_sb[K:128, :], 0.0)

            with (
                tc.tile_pool(name="wu_ps", bufs=1, space="PSUM") as wu_ps,
                tc.tile_pool(name="e_ps", bufs=1, space="PSUM") as e_ps,
                tc.tile_pool(name="sm_ps", bufs=1, space="PSUM") as sm_ps,
                tc.tile_pool(name="at_ps", bufs=1, space="PSUM") as at_ps,
            ):
                wp = wu_ps.tile([128, 128], F32)

                def keep_warm(n):
                    for _ in range(n):
                        nc.tensor.matmul(
                            wp[:], lhsT=wa[:], rhs=wa[:], start=True, stop=True
                        )

                keep_warm(6)

                # y weight tiles for the out-matmul: PE-transpose from yT,
                # 8 transposes per PSUM bank + one DVE copy per bank
                with tc.tile_pool(name="yt_ps", bufs=1, space="PSUM") as yt_ps:
                    for grp in range(SC // 8):
                        ypt = yt_ps.tile([K, 1024], BF16, tag="yt", name=f"yp{grp}")
                        for j in range(8):
                            g = grp * 8 + j
                            nc.tensor.transpose(
                                ypt[:, j * 128 : (j + 1) * 128], yT(g), ident
                            )
                        nc.vector.tensor_copy(
                            y_sb[0:K, grp * 1024 : (grp + 1) * 1024], ypt[:]
                        )

                # energy: e^T[64k, 512c] accumulated over the 32 s-chunks;
                # yT chunk is the stationary operand, xT streams 512 cols
                e_t = e_ps.tile([K, C], F32)
                for g in range(SC):
                    nc.tensor.matmul(
                        e_t[:],
                        lhsT=yT(g),
                        rhs=xT(g),
                        start=(g == 0),
                        stop=(g == SC - 1),
                    )
                    if g % 4 == 1:
                        # insurance against HAM idle-windows if the DMA
                        # stream runs ahead gaps: pinned behind chunk g
                        nc.tensor.matmul(
                            wp[:],
                            lhsT=wa[:],
                            rhs=xy[:, YW + g * 512 : YW + g * 512 + 128],
                            start=True,
                            stop=True,
                        )

                # softmax, wave-pipelined across the four c-chunks, with
                # keep-warm matmuls pinned behind softmax tiles filling
                # the PE windows: E^T -> SBUF (per-cc ScalarE copies), PE
                # transpose to [c,k], DVE min, ScalarE Exp(bias=min,
                # accum sum), DVE reciprocal, DVE (p*rcp*scale) -> bf16,
                # PE transpose back, ScalarE copy to attT.  scale folds
                # into the weights here.
                eT_sb = etp.tile([K, C], F32)
                for cc in range(CC):
                    nc.scalar.activation(
                        out=eT_sb[:, cc * 128 : (cc + 1) * 128],
                        in_=e_t[:, cc * 128 : (cc + 1) * 128],
                        func=AF.Copy,
                    )
                ecc_all = sm_ps.tile([128, CC * K], F32, tag="sm")
                eccs = [ecc_all[:, cc * K : (cc + 1) * K] for cc in range(CC)]
                for cc in range(CC):
                    nc.tensor.transpose(
                        eccs[cc], eT_sb[:, cc * 128 : (cc + 1) * 128],
                        ident_f[0:K, 0:K],
                    )
                for cc in range(CC):
                    nc.tensor.matmul(
                        wp[:],
                        lhsT=wa_f[0:K, :],
                        rhs=eT_sb[:, cc * 128 : (cc + 1) * 128],
                        start=True,
                        stop=True,
                    )
                rmins = []
                for cc in range(CC):
                    rmin = smp.tile([128, 1], F32, tag="sm")
                    nc.vector.tensor_reduce(
                        out=rmin, in_=eccs[cc], axis=AX.X, op=OP.min
                    )
                    rmins.append(rmin)
                rcps, p_ts = [], []
                for cc in range(CC):
                    p_t = pp.tile([128, K], F32, tag="p")
                    ssum = smp.tile([128, 1], F32, tag="sm")
                    nc.scalar.activation(
                        out=p_t[:],
                        in_=eccs[cc],
                        func=AF.Exp,
                        bias=rmins[cc],
                        scale=-1.0,
                        accum_out=ssum,
                    )
                    rcp = smp.tile([128, 1], F32, tag="sm")
                    nc.vector.reciprocal(out=rcp, in_=ssum)
                    p_ts.append(p_t)
                    rcps.append(rcp)
                for cc in range(CC):
                    nc.tensor.matmul(
                        wp[:, 0:K],
                        lhsT=wa_f[:],
                        rhs=p_ts[cc][:],
                        start=True,
                        stop=True,
                    )
                at_all = at_ps.tile([K, C], BF16, tag="at")
                for cc in range(CC):
                    attbf = pp.tile([128, K], BF16, tag="att")
                    nc.vector.tensor_scalar(
                        out=attbf[:],
                        in0=p_ts[cc][:],
                        scalar1=rcps[cc],
                        scalar2=scale_sb,
                        op0=OP.mult,
                        op1=OP.mult,
                    )
                    nc.tensor.matmul(
                        wp[:, 0:K], lhsT=wa[:], rhs=attbf[:], start=True, stop=True
                    )
                    nc.tensor.transpose(
                        at_all[:, cc * 128 : (cc + 1) * 128], attbf[:], ident
                    )
                    nc.scalar.activation(
                        out=attT_sb[0:K, cc * 128 : (cc + 1) * 128],
                        in_=at_all[:, cc * 128 : (cc + 1) * 128],
                        func=AF.Copy,
                    )

                # out phase: outT[128s, 512c] = y_g^T @ att^T + xT_g,
                # stored in 512KB groups of 4 chunks.  P chunks run the
                # identity-matmul (deps: xT only) FIRST so it fills the
                # softmax latency window, then accumulate the attention
                # matmul on top and ScalarE-copies to SBUF; V chunks run
                # the attention matmul (cc-sliced for the first groups,
                # starting as attT slices land) and DVE-add the residual
                # from PSUM.  16 chunks each keeps DVE/ScalarE/PE all
                # under the store-stream time.
                with tc.tile_pool(name="o_ps", bufs=4, space="PSUM") as o_ps:
                    for q4 in range(SC // 4):
                        res = resp.tile(
                            [128, 2048], BF16, name=f"res{q4}", tag="res"
                        )
                        for j in range(4):
                            g = 4 * q4 + j
                            o_t = o_ps.tile(
                                [128, C], F32, name=f"o_t{g}", tag="o_t"
                            )
                            if g % 2 == 0:  # P: identity first, attn second
                                nc.tensor.matmul(
                                    o_t[:],
                                    lhsT=ident,
                                    rhs=xT(g),
                                    start=True,
                                    stop=False,
                                )
                                nc.tensor.matmul(
                                    o_t[:],
                                    lhsT=y_sb[:, g * 128 : (g + 1) * 128],
                                    rhs=attT_sb[:],
                                    start=False,
                                    stop=True,
                                )
                                nc.scalar.activation(
                                    out=res[:, j * 512 : (j + 1) * 512],
                                    in_=o_t[:],
                                    func=AF.Copy,
                                )
                            else:  # V: attention matmul + DVE residual add
                                if g < 8:
                                    for cc in range(CC):
                                        nc.tensor.matmul(
                                            o_t[:, cc * 128 : (cc + 1) * 128],
                                            lhsT=y_sb[:, g * 128 : (g + 1) * 128],
                                            rhs=attT_sb[:, cc * 128 : (cc + 1) * 128],
                                            start=True,
                                            stop=True,
                                        )
                                else:
                                    nc.tensor.matmul(
                                        o_t[:],
                                        lhsT=y_sb[:, g * 128 : (g + 1) * 128],
                                        rhs=attT_sb[:],
                                        start=True,
                                        stop=True,
                                    )
                                nc.vector.tensor_add(
                                    res[:, j * 512 : (j + 1) * 512],
                                    xT(g),
                                    o_t[:],
                                )
                        nc.sync.dma_start(
                            out=o_d[q4 * 128 : (q4 + 1) * 128, :], in_=res[:]
                        )
    nc.compile()
    return nc


def _get_program():
    if "nc" not in _CACHE:
        _CACHE["nc"] = _build_program()
    return _CACHE["nc"]


def kernel(x, y, scale):
    from concourse import bass2jax

    nc = _get_program()
    x = np.asarray(x, dtype=np.float32).reshape(N, C, S)
    y = np.asarray(y, dtype=np.float32).reshape(N, K, S)
    scale = np.ascontiguousarray(np.asarray(scale, dtype=np.float32)).reshape(1)

    xy = _pack_inputs(x, y)
    in_maps = [{"xyT": xy[i], "scale": scale} for i in range(N)]
    results = bass2jax.run_bass_via_pjrt(nc, in_maps, n_cores=N)
    o = np.stack([np.asarray(results[i]["out"]) for i in range(N)])
    return _unpack_out(o).reshape(N, C, H, W)


# revision 29
# speedup vs baseline: 1.1623x; 1.0384x over previous
"""CCAMDec (channel-attention decoder) Trainium2 Bass kernel.

Data-parallel over batch N=8 across 8 NeuronCores (one batch per core).
Per core (C=512, K=64, HW=4096):
  energy[c,k]   = sum_s x[c,s] * y[k,s]         (bf16 matmul, fp32 accum)
  att[c,k]      = softmax_k(max_k(E) - E)       (== exp(min_k(E)-E)/sum)
  out[c,s]      = x[c,s] + scale * sum_k att[c,k] y[k,s]

Everything crosses HBM in bf16 with the s-contraction layout prepared on
the host (DMA is the roofline; ~8.9MB/core):
  xyT [128, 2048+16384]: cols [64g:64g+64]         = y[:, 128g+p].T (yT)
                         cols [2048+512g:...+512]  = x[:, 128g+p].T (xT)
  out [1024, 2048]: rows [128q:128q+128] cols 512j+c = outT[128(4q+j)+p, c]
One input tensor, loaded in three wide column chunks: HBM *reads* pay a
~150ns per-descriptor turnaround, so per-partition descriptor runs must
be long (12/16/8KB here ~= 75-83% of line rate; a 1KB-run tail chunk
would run at ~20%).

Energy accumulates as e^T[64k, 512c] (yT chunks stationary, xT streamed)
so no transposes sit on the load critical path.  y's [64,128] out-matmul
weight tiles are PE-transposed from yT during the load window (8 per
PSUM bank, one DVE copy per bank).  y/attT are zero-padded to 128
partitions: P=64 matmuls do not engage the PE HAM clock-unthrottle, and
a 1.2GHz out phase costs ~10us.  Dummy keep-warm matmuls pinned behind
softmax tiles bridge the HAM busy-window through the softmax latency
chain, and the P-chunk identity-matmuls (residual add of xT into PSUM,
commutative with the attention matmul) pre-run there too.

The residual is exact for scale=0: scale folds into the attention
weights, and the output equals bf16(x) bit-for-bit.
"""

import numpy as np
import ml_dtypes

N, C, K, H, W = 8, 512, 64, 64, 64
S = H * W          # 4096
SC = S // 128      # 32 s-chunks of 128
CC = C // 128      # 4 c-chunks of 128
YW = SC * K        # 2048 cols of yT at the head of xyT
BF = ml_dtypes.bfloat16

_CACHE = {}


def _pack_inputs(x, y):
    # x [n, C, S], y [n, K, S] -> xyT [n, 128, YW + SC*512] bf16
    n = x.shape[0]
    xb = x.reshape(n, C, SC, 128).astype(BF)
    xT = np.ascontiguousarray(xb.transpose(0, 3, 2, 1)).reshape(n, 128, SC * C)
    yb = y.reshape(n, K, SC, 128).astype(BF)
    yT = np.ascontiguousarray(yb.transpose(0, 3, 2, 1)).reshape(n, 128, YW)
    return np.concatenate([yT, xT], axis=2)


def _unpack_out(o):
    # o [n, 1024, 2048] bf16 -> [n, C, S] fp32
    n = o.shape[0]
    o = o.reshape(n, 8, 128, 4, 512).transpose(0, 1, 3, 2, 4).reshape(n, S, C)
    return np.ascontiguousarray(o.transpose(0, 2, 1)).astype(np.float32)


def _build_program():
    import concourse.tile as tile
    from concourse import bacc, mybir
    from concourse.masks import make_identity

    F32 = mybir.dt.float32
    BF16 = mybir.dt.bfloat16
    AX = mybir.AxisListType
    OP = mybir.AluOpType
    AF = mybir.ActivationFunctionType

    nc = bacc.Bacc("TRN2", target_bir_lowering=False, debug=False)
    xy_d = nc.dram_tensor("xyT", [128, YW + SC * 512], BF16, kind="ExternalInput")
    s_d = nc.dram_tensor("scale", [1], F32, kind="ExternalInput")
    o_d = nc.dram_tensor("out", [S // 4, 2048], BF16, kind="ExternalOutput")

    with tile.TileContext(nc) as tc:
        with (
            tc.tile_pool(name="const", bufs=1) as const,
            tc.tile_pool(name="xyp", bufs=1) as xyp,
            tc.tile_pool(name="ysbp", bufs=1) as ysbp,
            tc.tile_pool(name="etp", bufs=1) as etp,
            tc.tile_pool(name="smp", bufs=12) as smp,
            tc.tile_pool(name="pp", bufs=4) as pp,
            tc.tile_pool(name="attp", bufs=1) as attp,
            tc.tile_pool(name="resp", bufs=4) as resp,
        ):
            # yT first (small: feeds the y-transposes and energy weights
            # immediately), then x in ramping chunks: early arrivals keep
            # PE fed from ~5us on, wide middle chunks keep the HBM read
            # descriptors long
            xy = xyp.tile([128, YW + SC * 512], BF16)
            for a, b in ((0, 2048), (2048, 6144), (6144, 14336), (14336, 18432)):
                nc.sync.dma_start(out=xy[:, a:b], in_=xy_d[:, a:b])

            def xT(g):
                return xy[:, YW + g * 512 : YW + (g + 1) * 512]

            def yT(g):
                return xy[:, g * 64 : (g + 1) * 64]

            ident = const.tile([128, 128], BF16)
            make_identity(nc, ident)
            ident_f = const.tile([128, 128], F32)
            make_identity(nc, ident_f)

            scale_sb = const.tile([128, 1], F32)
            nc.gpsimd.dma_start(out=scale_sb, in_=s_d[:].to_broadcast([128, 1]))

            # prewarm both ScalarE LUTs (Exp and Copy) in the DMA-ramp head
            warm_in = const.tile([128, 1], F32)
            nc.vector.memset(warm_in, 0.0)
            warm = const.tile([128, 1], F32)
            nc.scalar.activation(out=warm, in_=warm_in, func=AF.Exp)
            warm2 = const.tile([128, 1], F32)
            nc.scalar.activation(out=warm2, in_=warm_in, func=AF.Copy)

            wa = const.tile([128, 128], BF16)
            nc.vector.memset(wa, 0.0)
            wa_f = const.tile([128, 128], F32)
            nc.vector.memset(wa_f, 0.0)

            # y and attT zero-padded to 128 partitions (P=128 matmuls keep
            # the PE HAM engaged; rows 64-127 contribute nothing)
            y_sb = ysbp.tile([128, S], BF16)
            nc.gpsimd.memset(y_sb[K:128, :], 0.0)
            attT_sb = attp.tile([128, C], BF16)
            nc.gpsimd.memset(attT_sb[K:128, :], 0.0)

            with (
                tc.tile_pool(name="wu_ps", bufs=1, space="PSUM") as wu_ps,
                tc.tile_pool(name="e_ps", bufs=1, space="PSUM") as e_ps,
                tc.tile_pool(name="sm_ps", bufs=1, space="PSUM") as sm_ps,
                tc.tile_pool(name="at_ps", bufs=1, space="PSUM") as at_ps,
            ):
                wp = wu_ps.tile([128, 128], F32)

                def keep_warm(n):
                    for _ in range(n):
                        nc.tensor.matmul(
                            wp[:], lhsT=wa[:], rhs=wa[:], start=True, stop=True
                        )

                keep_warm(6)

                # y weight tiles for the out-matmul: PE-transpose from yT,
                # 8 transposes per PSUM bank + one DVE copy per bank
                with tc.tile_pool(name="yt_ps", bufs=1, space="PSUM") as yt_ps:
                    for grp in range(SC // 8):
                        ypt = yt_ps.tile([K, 1024], BF16, tag="yt", name=f"yp{grp}")
                        for j in range(8):
                            g = grp * 8 + j
                            nc.tensor.transpose(
                                ypt[:, j * 128 : (j + 1) * 128], yT(g), ident
                            )
                        nc.vector.tensor_copy(
                            y_sb[0:K, grp * 1024 : (grp + 1) * 1024], ypt[:]
                        )

                # energy: e^T[64k, 512c] accumulated over the 32 s-chunks;
                # yT chunk is the stationary operand, xT streams 512 cols
                e_t = e_ps.tile([K, C], F32)
                for g in range(SC):
                    nc.tensor.matmul(
                        e_t[:],
                        lhsT=yT(g),
                        rhs=xT(g),
                        start=(g == 0),
                        stop=(g == SC - 1),
                    )
                    if g % 4 == 1:
                        # insurance against HAM idle-windows if the DMA
                        # stream runs ahead gaps: pinned behind chunk g
                        nc.tensor.matmul(
                            wp[:],
                            lhsT=wa[:],
                            rhs=xy[:, YW + g * 512 : YW + g * 512 + 128],
                            start=True,
                            stop=True,
                        )

                # softmax, wave-pipelined across the four c-chunks, with
                # keep-warm matmuls pinned behind softmax tiles filling
                # the PE windows: E^T -> SBUF (per-cc ScalarE copies), PE
                # transpose to [c,k], DVE min, ScalarE Exp(bias=min,
                # accum sum), DVE reciprocal, DVE (p*rcp*scale) -> bf16,
                # PE transpose back, ScalarE copy to attT.  scale folds
                # into the weights here.
                eT_sb = etp.tile([K, C], F32)
                for cc in range(CC):
                    nc.scalar.activation(
                        out=eT_sb[:, cc * 128 : (cc + 1) * 128],
                        in_=e_t[:, cc * 128 : (cc + 1) * 128],
                        func=AF.Copy,
                    )
                ecc_all = sm_ps.tile([128, CC * K], F32, tag="sm")
                eccs = [ecc_all[:, cc * K : (cc + 1) * K] for cc in range(CC)]
                for cc in range(CC):
                    nc.tensor.transpose(
                        eccs[cc], eT_sb[:, cc * 128 : (cc + 1) * 128],
                        ident_f[0:K, 0:K],
                    )
                for cc in range(CC):
                    nc.tensor.matmul(
                        wp[:],
                        lhsT=wa_f[0:K, :],
                        rhs=eT_sb[:, cc * 128 : (cc + 1) * 128],
                        start=True,
                        stop=True,
                    )
                rmins = []
                for cc in range(CC):
                    rmin = smp.tile([128, 1], F32, tag="sm")
                    nc.vector.tensor_reduce(
                        out=rmin, in_=eccs[cc], axis=AX.X, op=OP.min
                    )
                    rmins.append(rmin)
                rcps, p_ts = [], []
                for cc in range(CC):
                    p_t = pp.tile([128, K], F32, tag="p")
                    ssum = smp.tile([128, 1], F32, tag="sm")
                    nc.scalar.activation(
                        out=p_t[:],
                        in_=eccs[cc],
                        func=AF.Exp,
                        bias=rmins[cc],
                        scale=-1.0,
                        accum_out=ssum,
                    )
                    rcp = smp.tile([128, 1], F32, tag="sm")
                    nc.vector.reciprocal(out=rcp, in_=ssum)
                    p_ts.append(p_t)
                    rcps.append(rcp)
                for cc in range(CC):
                    nc.tensor.matmul(
                        wp[:, 0:K],
                        lhsT=wa_f[:],
                        rhs=p_ts[cc][:],
                        start=True,
                        stop=True,
                    )
                at_all = at_ps.tile([K, C], BF16, tag="at")
                for cc in range(CC):
                    attbf = pp.tile([128, K], BF16, tag="att")
                    nc.vector.tensor_scalar(
                        out=attbf[:],
                        in0=p_ts[cc][:],
                        scalar1=rcps[cc],
                        scalar2=scale_sb,
                        op0=OP.mult,
                        op1=OP.mult,
                    )
                    nc.tensor.matmul(
                        wp[:, 0:K], lhsT=wa[:], rhs=attbf[:], start=True, stop=True
                    )
                    nc.tensor.transpose(
                        at_all[:, cc * 128 : (cc + 1) * 128], attbf[:], ident
                    )
                    nc.scalar.activation(
                        out=attT_sb[0:K, cc * 128 : (cc + 1) * 128],
                        in_=at_all[:, cc * 128 : (cc + 1) * 128],
                        func=AF.Copy,
                    )

                # out phase: outT[128s, 512c] = y_g^T @ att^T + xT_g,
                # stored in 512KB groups of 4 chunks.  P chunks run the
                # identity-matmul (deps: xT only) FIRST so it fills the
                # softmax latency window, then accumulate the attention
                # matmul on top and ScalarE-copies to SBUF; V chunks run
                # the attention matmul (cc-sliced for the first groups,
                # starting as attT slices land) and DVE-add the residual
                # from PSUM.  16 chunks each keeps DVE/ScalarE/PE all
                # under the store-stream time.
                with tc.tile_pool(name="o_ps", bufs=4, space="PSUM") as o_ps:
                    for q4 in range(SC // 4):
                        res = resp.tile(
                            [128, 2048], BF16, name=f"res{q4}", tag="res"
                        )
                        for j in range(4):
                            g = 4 * q4 + j
                            o_t = o_ps.tile(
                                [128, C], F32, name=f"o_t{g}", tag="o_t"
                            )
                            if g % 2 == 0:  # P: identity first, attn second
                                nc.tensor.matmul(
                                    o_t[:],
                                    lhsT=ident,
                                    rhs=xT(g),
                                    start=True,
                                    stop=False,
                                )
                                nc.tensor.matmul(
                                    o_t[:],
                                    lhsT=y_sb[:, g * 128 : (g + 1) * 128],
                                    rhs=attT_sb[:],
                                    start=False,
                                    stop=True,
                                )
                                nc.scalar.activation(
                                    out=res[:, j * 512 : (j + 1) * 512],
                                    in_=o_t[:],
                                    func=AF.Copy,
                                )
                            else:  # V: attention matmul + DVE residual add
                                if g < 8:
                                    for cc in range(CC):
                                        nc.tensor.matmul(
                                            o_t[:, cc * 128 : (cc + 1) * 128],
                                            lhsT=y_sb[:, g * 128 : (g + 1) * 128],
                                            rhs=attT_sb[:, cc * 128 : (cc + 1) * 128],
                                            start=True,
                                            stop=True,
                                        )
                                else:
                                    nc.tensor.matmul(
                                        o_t[:],
                                        lhsT=y_sb[:, g * 128 : (g + 1) * 128],
                                        rhs=attT_sb[:],
                                        start=True,
                                        stop=True,
                                    )
                                nc.vector.tensor_add(
                                    res[:, j * 512 : (j + 1) * 512],
                                    xT(g),
                                    o_t[:],
                                )
                        nc.sync.dma_start(
                            out=o_d[q4 * 128 : (q4 + 1) * 128, :], in_=res[:]
                        )
    nc.compile()
    return nc


def _get_program():
    if "nc" not in _CACHE:
        _CACHE["nc"] = _build_program()
    return _CACHE["nc"]


def kernel(x, y, scale):
    from concourse import bass2jax

    nc = _get_program()
    x = np.asarray(x, dtype=np.float32).reshape(N, C, S)
    y = np.asarray(y, dtype=np.float32).reshape(N, K, S)
    scale = np.ascontiguousarray(np.asarray(scale, dtype=np.float32)).reshape(1)

    xy = _pack_inputs(x, y)
    in_maps = [{"xyT": xy[i], "scale": scale} for i in range(N)]
    results = bass2jax.run_bass_via_pjrt(nc, in_maps, n_cores=N)
    o = np.stack([np.asarray(results[i]["out"]) for i in range(N)])
    return _unpack_out(o).reshape(N, C, H, W)


# revision 30
# speedup vs baseline: 1.2089x; 1.0401x over previous
"""CCAMDec (channel-attention decoder) Trainium2 Bass kernel.

Data-parallel over batch N=8 across 8 NeuronCores (one batch per core).
Per core (C=512, K=64, HW=4096):
  energy[c,k]   = sum_s x[c,s] * y[k,s]         (bf16 matmul, fp32 accum)
  att[c,k]      = softmax_k(max_k(E) - E)       (== exp(min_k(E)-E)/sum)
  out[c,s]      = x[c,s] + scale * sum_k att[c,k] y[k,s]

Everything crosses HBM in bf16 with the s-contraction layout prepared on
the host (DMA is the roofline; ~8.9MB/core):
  xyT [128, 2048+16384]: cols [64g:64g+64]         = y[:, 128g+p].T (yT)
                         cols [2048+512g:...+512]  = x[:, 128g+p].T (xT)
  out [1024, 2048]: rows [128q:128q+128] cols 512j+c = outT[128(4q+j)+p, c]
One input tensor, loaded in three wide column chunks: HBM *reads* pay a
~150ns per-descriptor turnaround, so per-partition descriptor runs must
be long (12/16/8KB here ~= 75-83% of line rate; a 1KB-run tail chunk
would run at ~20%).

Energy accumulates as e^T[64k, 512c] (yT chunks stationary, xT streamed)
so no transposes sit on the load critical path.  y's [64,128] out-matmul
weight tiles are PE-transposed from yT during the load window (8 per
PSUM bank, one DVE copy per bank).  y/attT are zero-padded to 128
partitions: P=64 matmuls do not engage the PE HAM clock-unthrottle, and
a 1.2GHz out phase costs ~10us.  Dummy keep-warm matmuls pinned behind
softmax tiles bridge the HAM busy-window through the softmax latency
chain, and the P-chunk identity-matmuls (residual add of xT into PSUM,
commutative with the attention matmul) pre-run there too.

The residual is exact for scale=0: scale folds into the attention
weights, and the output equals bf16(x) bit-for-bit.
"""

import numpy as np
import ml_dtypes

N, C, K, H, W = 8, 512, 64, 64, 64
S = H * W          # 4096
SC = S // 128      # 32 s-chunks of 128
CC = C // 128      # 4 c-chunks of 128
YW = SC * K        # 2048 cols of yT at the head of xyT
BF = ml_dtypes.bfloat16

_CACHE = {}


def _pack_inputs(x, y):
    # x [n, C, S], y [n, K, S] -> xyT [n, 128, YW + SC*512] bf16, y bf16
    n = x.shape[0]
    xb = x.reshape(n, C, SC, 128).astype(BF)
    xT = np.ascontiguousarray(xb.transpose(0, 3, 2, 1)).reshape(n, 128, SC * C)
    yb = y.reshape(n, K, SC, 128).astype(BF)
    yT = np.ascontiguousarray(yb.transpose(0, 3, 2, 1)).reshape(n, 128, YW)
    return np.concatenate([yT, xT], axis=2), np.ascontiguousarray(y.astype(BF))


def _unpack_out(o):
    # o [n, 1024, 2048] bf16 -> [n, C, S] fp32
    n = o.shape[0]
    o = o.reshape(n, 8, 128, 4, 512).transpose(0, 1, 3, 2, 4).reshape(n, S, C)
    return np.ascontiguousarray(o.transpose(0, 2, 1)).astype(np.float32)


def _build_program():
    import concourse.tile as tile
    from concourse import bacc, mybir
    from concourse.masks import make_identity

    F32 = mybir.dt.float32
    BF16 = mybir.dt.bfloat16
    AX = mybir.AxisListType
    OP = mybir.AluOpType
    AF = mybir.ActivationFunctionType

    nc = bacc.Bacc("TRN2", target_bir_lowering=False, debug=False)
    xy_d = nc.dram_tensor("xyT", [128, YW + SC * 512], BF16, kind="ExternalInput")
    y_d = nc.dram_tensor("y", [K, S], BF16, kind="ExternalInput")
    s_d = nc.dram_tensor("scale", [1], F32, kind="ExternalInput")
    o_d = nc.dram_tensor("out", [S // 4, 2048], BF16, kind="ExternalOutput")

    with tile.TileContext(nc) as tc:
        with (
            tc.tile_pool(name="const", bufs=1) as const,
            tc.tile_pool(name="xyp", bufs=1) as xyp,
            tc.tile_pool(name="ysbp", bufs=1) as ysbp,
            tc.tile_pool(name="etp", bufs=1) as etp,
            tc.tile_pool(name="smp", bufs=12) as smp,
            tc.tile_pool(name="pp", bufs=4) as pp,
            tc.tile_pool(name="attp", bufs=1) as attp,
            tc.tile_pool(name="resp", bufs=4) as resp,
        ):
            # yT first (small: feeds the y-transposes and energy weights
            # immediately), then x in ramping chunks: early arrivals keep
            # PE fed from ~5us on, wide middle chunks keep the HBM read
            # descriptors long
            xy = xyp.tile([128, YW + SC * 512], BF16)
            for a, b in ((0, 2048), (2048, 6144), (6144, 14336), (14336, 18432)):
                nc.sync.dma_start(out=xy[:, a:b], in_=xy_d[:, a:b])
            # y-normal (out-matmul weights) last: needed only after the
            # softmax, so its half-width drain hides behind that window

            def xT(g):
                return xy[:, YW + g * 512 : YW + (g + 1) * 512]

            def yT(g):
                return xy[:, g * 64 : (g + 1) * 64]

            ident = const.tile([128, 128], BF16)
            make_identity(nc, ident)
            ident_f = const.tile([128, 128], F32)
            make_identity(nc, ident_f)

            scale_sb = const.tile([128, 1], F32)
            nc.gpsimd.dma_start(out=scale_sb, in_=s_d[:].to_broadcast([128, 1]))

            # prewarm both ScalarE LUTs (Exp and Copy) in the DMA-ramp head
            warm_in = const.tile([128, 1], F32)
            nc.vector.memset(warm_in, 0.0)
            warm = const.tile([128, 1], F32)
            nc.scalar.activation(out=warm, in_=warm_in, func=AF.Exp)
            warm2 = const.tile([128, 1], F32)
            nc.scalar.activation(out=warm2, in_=warm_in, func=AF.Copy)

            wa = const.tile([128, 128], BF16)
            nc.vector.memset(wa, 0.0)
            wa_f = const.tile([128, 128], F32)
            nc.vector.memset(wa_f, 0.0)

            # y and attT zero-padded to 128 partitions (P=128 matmuls keep
            # the PE HAM engaged; rows 64-127 contribute nothing)
            y_sb = ysbp.tile([128, S], BF16)
            nc.gpsimd.memset(y_sb[K:128, :], 0.0)
            nc.sync.dma_start(out=y_sb[0:K, :], in_=y_d[:])
            attT_sb = attp.tile([128, C], BF16)
            nc.gpsimd.memset(attT_sb[K:128, :], 0.0)

            with (
                tc.tile_pool(name="wu_ps", bufs=1, space="PSUM") as wu_ps,
                tc.tile_pool(name="e_ps", bufs=1, space="PSUM") as e_ps,
                tc.tile_pool(name="sm_ps", bufs=1, space="PSUM") as sm_ps,
                tc.tile_pool(name="at_ps", bufs=1, space="PSUM") as at_ps,
            ):
                wp = wu_ps.tile([128, 128], F32)

                def keep_warm(n):
                    for _ in range(n):
                        nc.tensor.matmul(
                            wp[:], lhsT=wa[:], rhs=wa[:], start=True, stop=True
                        )

                keep_warm(6)

                # energy: e^T[64k, 512c] accumulated over the 32 s-chunks;
                # yT chunk is the stationary operand, xT streams 512 cols
                e_t = e_ps.tile([K, C], F32)
                for g in range(SC):
                    nc.tensor.matmul(
                        e_t[:],
                        lhsT=yT(g),
                        rhs=xT(g),
                        start=(g == 0),
                        stop=(g == SC - 1),
                    )
                    if g % 4 == 1:
                        # insurance against HAM idle-windows if the DMA
                        # stream runs ahead gaps: pinned behind chunk g
                        nc.tensor.matmul(
                            wp[:],
                            lhsT=wa[:],
                            rhs=xy[:, YW + g * 512 : YW + g * 512 + 128],
                            start=True,
                            stop=True,
                        )

                # softmax, wave-pipelined across the four c-chunks, with
                # keep-warm matmuls pinned behind softmax tiles filling
                # the PE windows: E^T -> SBUF (per-cc ScalarE copies), PE
                # transpose to [c,k], DVE min, ScalarE Exp(bias=min,
                # accum sum), DVE reciprocal, DVE (p*rcp*scale) -> bf16,
                # PE transpose back, ScalarE copy to attT.  scale folds
                # into the weights here.
                eT_sb = etp.tile([K, C], F32)
                for cc in range(CC):
                    nc.scalar.activation(
                        out=eT_sb[:, cc * 128 : (cc + 1) * 128],
                        in_=e_t[:, cc * 128 : (cc + 1) * 128],
                        func=AF.Copy,
                    )
                ecc_all = sm_ps.tile([128, CC * K], F32, tag="sm")
                eccs = [ecc_all[:, cc * K : (cc + 1) * K] for cc in range(CC)]
                for cc in range(CC):
                    nc.tensor.transpose(
                        eccs[cc], eT_sb[:, cc * 128 : (cc + 1) * 128],
                        ident_f[0:K, 0:K],
                    )
                for cc in range(CC):
                    nc.tensor.matmul(
                        wp[:],
                        lhsT=wa_f[0:K, :],
                        rhs=eT_sb[:, cc * 128 : (cc + 1) * 128],
                        start=True,
                        stop=True,
                    )
                rmins = []
                for cc in range(CC):
                    rmin = smp.tile([128, 1], F32, tag="sm")
                    nc.vector.tensor_reduce(
                        out=rmin, in_=eccs[cc], axis=AX.X, op=OP.min
                    )
                    rmins.append(rmin)
                rcps, p_ts = [], []
                for cc in range(CC):
                    p_t = pp.tile([128, K], F32, tag="p")
                    ssum = smp.tile([128, 1], F32, tag="sm")
                    nc.scalar.activation(
                        out=p_t[:],
                        in_=eccs[cc],
                        func=AF.Exp,
                        bias=rmins[cc],
                        scale=-1.0,
                        accum_out=ssum,
                    )
                    rcp = smp.tile([128, 1], F32, tag="sm")
                    nc.vector.reciprocal(out=rcp, in_=ssum)
                    p_ts.append(p_t)
                    rcps.append(rcp)
                for cc in range(CC):
                    nc.tensor.matmul(
                        wp[:, 0:K],
                        lhsT=wa_f[:],
                        rhs=p_ts[cc][:],
                        start=True,
                        stop=True,
                    )
                at_all = at_ps.tile([K, C], BF16, tag="at")
                for cc in range(CC):
                    attbf = pp.tile([128, K], BF16, tag="att")
                    nc.vector.tensor_scalar(
                        out=attbf[:],
                        in0=p_ts[cc][:],
                        scalar1=rcps[cc],
                        scalar2=scale_sb,
                        op0=OP.mult,
                        op1=OP.mult,
                    )
                    nc.tensor.matmul(
                        wp[:, 0:K], lhsT=wa[:], rhs=attbf[:], start=True, stop=True
                    )
                    nc.tensor.transpose(
                        at_all[:, cc * 128 : (cc + 1) * 128], attbf[:], ident
                    )
                    nc.scalar.activation(
                        out=attT_sb[0:K, cc * 128 : (cc + 1) * 128],
                        in_=at_all[:, cc * 128 : (cc + 1) * 128],
                        func=AF.Copy,
                    )

                # out phase: outT[128s, 512c] = y_g^T @ att^T + xT_g,
                # stored in 512KB groups of 4 chunks.  P chunks run the
                # identity-matmul (deps: xT only) FIRST so it fills the
                # softmax latency window, then accumulate the attention
                # matmul on top and ScalarE-copies to SBUF; V chunks run
                # the attention matmul (cc-sliced for the first groups,
                # starting as attT slices land) and DVE-add the residual
                # from PSUM.  16 chunks each keeps DVE/ScalarE/PE all
                # under the store-stream time.
                with tc.tile_pool(name="o_ps", bufs=4, space="PSUM") as o_ps:
                    for q4 in range(SC // 4):
                        res = resp.tile(
                            [128, 2048], BF16, name=f"res{q4}", tag="res"
                        )
                        for j in range(4):
                            g = 4 * q4 + j
                            o_t = o_ps.tile(
                                [128, C], F32, name=f"o_t{g}", tag="o_t"
                            )
                            if g % 2 == 0:  # P: identity first, attn second
                                nc.tensor.matmul(
                                    o_t[:],
                                    lhsT=ident,
                                    rhs=xT(g),
                                    start=True,
                                    stop=False,
                                )
                                nc.tensor.matmul(
                                    o_t[:],
                                    lhsT=y_sb[:, g * 128 : (g + 1) * 128],
                                    rhs=attT_sb[:],
                                    start=False,
                                    stop=True,
                                )
                                nc.scalar.activation(
                                    out=res[:, j * 512 : (j + 1) * 512],
                                    in_=o_t[:],
                                    func=AF.Copy,
                                )
                            else:  # V: attention matmul + DVE residual add
                                if g < 8:
                                    for cc in range(CC):
                                        nc.tensor.matmul(
                                            o_t[:, cc * 128 : (cc + 1) * 128],
                                            lhsT=y_sb[:, g * 128 : (g + 1) * 128],
                                            rhs=attT_sb[:, cc * 128 : (cc + 1) * 128],
                                            start=True,
                                            stop=True,
                                        )
                                else:
                                    nc.tensor.matmul(
                                        o_t[:],
                                        lhsT=y_sb[:, g * 128 : (g + 1) * 128],
                                        rhs=attT_sb[:],
                                        start=True,
                                        stop=True,
                                    )
                                nc.vector.tensor_add(
                                    res[:, j * 512 : (j + 1) * 512],
                                    xT(g),
                                    o_t[:],
                                )
                        nc.sync.dma_start(
                            out=o_d[q4 * 128 : (q4 + 1) * 128, :], in_=res[:]
                        )
    nc.compile()
    return nc


def _get_program():
    if "nc" not in _CACHE:
        _CACHE["nc"] = _build_program()
    return _CACHE["nc"]


def kernel(x, y, scale):
    from concourse import bass2jax

    nc = _get_program()
    x = np.asarray(x, dtype=np.float32).reshape(N, C, S)
    y = np.asarray(y, dtype=np.float32).reshape(N, K, S)
    scale = np.ascontiguousarray(np.asarray(scale, dtype=np.float32)).reshape(1)

    xy, yn = _pack_inputs(x, y)
    in_maps = [{"xyT": xy[i], "y": yn[i], "scale": scale} for i in range(N)]
    results = bass2jax.run_bass_via_pjrt(nc, in_maps, n_cores=N)
    o = np.stack([np.asarray(results[i]["out"]) for i in range(N)])
    return _unpack_out(o).reshape(N, C, H, W)
